# revision 46
# baseline (speedup 1.0000x reference)
"""CaptchaCRNN Trainium2 kernel: 7 convs + 2 train-mode BN + maxpools + biLSTM.

Data-parallel over batch on 8 NeuronCores (8 images/core). BN batch stats are
globalized with pipelined AllReduces overlapped under the next conv. Convs run
in bf16 (1 cyc/row, FWL weight loads); pools drain PSUM via DVE/GPSIMD so the
scalar engine never bottlenecks; conv4 computes only the even rows that
survive its (1,2)x(2,2) maxpool.
"""
import sys

sys.path.insert(0, "/opt/trn_rl_repo")

import numpy as np
import ml_dtypes
import concourse.bass as bass
import concourse.bacc as bacc
import concourse.tile as tile
from concourse import masks
from concourse import mybir
from concourse import bass_utils

F32 = mybir.dt.float32
F16 = mybir.dt.float16
BF16 = mybir.dt.bfloat16
AF = mybir.ActivationFunctionType
ALU = mybir.AluOpType
AX = mybir.AxisListType

NCORES = 8
B = 8          # images per core
EPS = 1e-5
INV_N = 1.0 / (64 * 8 * 32)   # BN normalizer: full batch 64 x H8 x W32

# 4H gate permutation: torch order [i,f,g,o] -> compute order [i,f,o,g]
PERM4H = np.r_[0:512, 768:1024, 512:768]



def _ap(obj, offset, dims):
    base = obj if isinstance(obj, bass.AP) else obj[:]
    return bass.AP(tensor=base.tensor, offset=base.offset + offset,
                   ap=[list(d) for d in dims])


def build(debug=False):
    nc = bacc.Bacc("TRN2", target_bir_lowering=False, debug=False,
                   enable_asserts=True, num_devices=NCORES)

    def din(name, shape, dt=F32):
        return nc.dram_tensor(name, list(shape), dt, kind="ExternalInput").ap()

    def dout(name, shape, dt=F32):
        return nc.dram_tensor(name, list(shape), dt, kind="ExternalOutput").ap()

    xim = din("xim", (B, 19, 64, 128), F16)   # im2col taps, w-half blocked
    w1x = din("w1x", (19, 128), F16)
    w2p = din("w2p", (3, 128, 128), F16)
    w2s = din("w2s", (3, 64, 128), F16)
    w3T = din("w3T", (1, 9, 128, 256), F16)
    w4T = din("w4T", (2, 9, 128, 256), F16)
    w5T = din("w5T", (2, 9, 128, 512), F16)
    w6T = din("w6T", (4, 9, 128, 512), F16)
    w7T = din("w7T", (4, 4, 128, 512), F16)
    b2 = din("b2", (128, 1))
    b3 = din("b3", (128, 2))
    b4 = din("b4", (128, 2))
    b5 = din("b5", (128, 4))
    b6 = din("b6", (128, 4))
    b7 = din("b7", (128, 4))
    gam = din("gam", (128, 4))
    bet = din("bet", (128, 4))
    wihT = din("wihT", (2, 8, 128, 1024), F16)
    whhT = din("whhT", (2, 2, 128, 1024), F16)
    lbias = din("lbias", (128, 2, 8))
    out = dout("out", (B, 15, 512), F16)

    dbg = {}
    if debug:
        dbg["a2"] = dout("dbg_a2", (128, 8, 16, 64), F16)
        dbg["a4"] = dout("dbg_a4", (128, 2, 8, 8, 32), F16)
        dbg["a5"] = dout("dbg_a5", (128, 4, 8, 8, 32), F16)
        dbg["c6p"] = dout("dbg_c6p", (128, 4, 8, 4, 16), F16)
        dbg["c7"] = dout("dbg_c7", (128, 4, 8, 3, 16), F16)
        dbg["xg"] = dout("dbg_xg", (128, 2, 8, 8, 15))
        dbg["hs"] = dout("dbg_hs", (128, 2, 2, 8, 15), F16)

    with tile.TileContext(nc) as tc:
        opened = []

        def popen(name, bufs, space="SBUF", side=None):
            cm = tc.tile_pool(name=name, bufs=bufs, space=space, side=side)
            p = cm.__enter__()
            p._cm = cm
            opened.append(p)
            return p

        def pclose(p):
            p._cm.__exit__(None, None, None)
            opened.remove(p)

        const = popen("const", 1, side="left")
        dram = popen("dram", 1, space="DRAM")

        # ---- constants ----
        b2_sb = const.tile([128, 1], F32)
        nc.sync.dma_start(out=b2_sb[:], in_=b2)
        b3_sb = const.tile([128, 2], F32)
        nc.sync.dma_start(out=b3_sb[:], in_=b3)
        b4_sb = const.tile([128, 2], F32)
        nc.sync.dma_start(out=b4_sb[:], in_=b4)
        b5_sb = const.tile([128, 4], F32)
        nc.sync.dma_start(out=b5_sb[:], in_=b5)
        b6_sb = const.tile([128, 4], F32)
        nc.sync.dma_start(out=b6_sb[:], in_=b6)
        b7_sb = const.tile([128, 4], F32)
        nc.sync.dma_start(out=b7_sb[:], in_=b7)
        gam_sb = const.tile([128, 4], F32)
        nc.sync.dma_start(out=gam_sb[:], in_=gam)
        bet_sb = const.tile([128, 4], F32)
        nc.sync.dma_start(out=bet_sb[:], in_=bet)
        eps_sb = const.tile([128, 1], F32)
        nc.vector.memset(eps_sb[:], EPS)
        ident16 = const.tile([128, 128], F16)
        masks.make_identity(nc, ident16[:])

        # ---- conv1+2 weights ----
        wA = popen("wA", 1, side="left")
        w1_sb = wA.tile([19, 128], F16)
        nc.sync.dma_start(out=w1_sb[:], in_=w1x)
        w2p_sb = wA.tile([128, 3, 128], F16)
        nc.gpsimd.dma_start(
            out=w2p_sb[:],
            in_=_ap(w2p, 0, [[128, 128], [128 * 128, 3], [1, 128]]))
        w2s_sb = wA.tile([128, 3, 128], F16)
        nc.gpsimd.dma_start(
            out=w2s_sb[64:128, :, :],
            in_=_ap(w2s, 0, [[128, 64], [64 * 128, 3], [1, 128]]))
        w3_sb = wA.tile([128, 9, 256], F16)
        nc.gpsimd.dma_start(
            out=w3_sb[:],
            in_=_ap(w3T, 0, [[256, 128], [128 * 256, 9], [1, 256]]))
        w4_sb = wA.tile([128, 2, 9, 256], F16)
        nc.gpsimd.dma_start(
            out=w4_sb[:],
            in_=_ap(w4T, 0, [[256, 128], [9 * 128 * 256, 2], [128 * 256, 9],
                             [1, 256]]))

        # ---- conv1 + a1b assembly + conv2: interleaved, multi-ring DMA ----
        # conv1 psum per mm [128, 4h, 128w]; p<64: ch at w<128, p>=64: w>=128
        xs_pool = popen("xs", 2, side="left")
        hmt_pool = popen("hmt", 4, side="left")
        ps1 = popen("ps1", 4, space="PSUM")
        ps2 = popen("ps2", 4, space="PSUM")
        a1b_pool = popen("a1b", 4, side="left")
        st2 = popen("st2", 3, side="left")
        w2m_pool = popen("w2m", 3, side="left")
        # prefetch w5, w6 during conv1-4 (gpsimd ring)
        wB = popen("wB", 1, side="right")
        w5_sb = wB.tile([128, 2, 9, 512], F16)
        nc.gpsimd.dma_start(
            out=w5_sb[:],
            in_=_ap(w5T, 0, [[512, 128], [9 * 128 * 512, 2], [128 * 512, 9],
                             [1, 512]]))
        w6_sb = wB.tile([128, 4, 9, 512], F16)   # DMA issued after conv1/2
        a2p_pool = popen("a2p", 8, side="right")
        a2p_t = [a2p_pool.tile([128, 18, 66], F16, tag="a2p", name=f"a2p{b}")
                 for b in range(B)]
        # border zeros once, on scalar (idle during conv1); columns are zeroed
        # 2-wide (interior neighbor is rewritten by later compute/DMA)
        for b in range(B):
            for r in (0, 17):
                nc.scalar.memzero(a2p_t[b][:, r, :])
            nc.scalar.memzero(a2p_t[b][:, :, 0:2])
            nc.scalar.memzero(a2p_t[b][:, :, 64:66])

        a1b_t = []   # ring of 4; borders zeroed once, interior rewritten
        wp_pool = popen("wp", 1, side="left")
        wp_t = []

        def conv1_setup(b):
            xsa = xs_pool.tile([19, 32, 128], F16, tag="xsa", bufs=2)
            xsb = xs_pool.tile([19, 32, 128], F16, tag="xsb", bufs=2)
            base = b * (19 * 64 * 128)
            nc.sync.dma_start(
                out=xsa[:], in_=_ap(xim, base, [[64 * 128, 19], [1, 32 * 128]]))
            nc.scalar.dma_start(
                out=xsb[:], in_=_ap(xim, base + 32 * 128,
                                    [[64 * 128, 19], [1, 32 * 128]]))
            if b < 4:
                a1b = a1b_pool.tile([128, 34, 130], F16, tag="a1b", bufs=4,
                                    name=f"a1b{b % 4}")
                a1b_t.append(a1b)
                for r in (0, 33):
                    nc.scalar.memzero(a1b[0:64, r, :])
                nc.scalar.memzero(a1b[64:128, 32:34, :])
                nc.scalar.memzero(a1b[:, :, 0:2])
                nc.scalar.memzero(a1b[:, :, 128:130])
            else:
                a1b_t.append(a1b_t[b - 4])
            hmi = wp_pool.tile([128, 32, 128], F16, tag="hmi", bufs=2,
                               name=f"hmi{b % 2}")
            wpt = wp_pool.tile([128, 32, 64], F16, tag="wp", bufs=3,
                               name=f"wp{b % 3}")
            wp_t.append((hmi, wpt))
            return xsa, xsb

        def conv1_mm(b, xsa, xsb, j):
            # one mm: rows 4j..4j+3; h-pair max -> hmi[:, 2j:2j+2, :]
            xs = xsa if j < 8 else xsb
            r0 = 4 * (j % 8)
            hmi = wp_t[b][0]
            pt = ps1.tile([128, 4, 128], F32, tag="p1", bufs=4)
            nc.tensor.matmul(pt[:], w1_sb[:], xs[:, r0:r0 + 4, :],
                             start=True, stop=True)
            s0 = hmt_pool.tile([128, 2, 128], F16, tag="s0", bufs=4)
            nc.scalar.activation(
                out=s0[:].rearrange("p a b -> p (a b)"), in_=pt[:, 0::2, :],
                func=AF.Identity, bias=0.0, scale=1.0)
            nc.vector.tensor_max(out=hmi[:, 2 * j:2 * j + 2, :],
                                 in0=pt[:, 1::2, :], in1=s0[:])

        def conv1_pool(b):
            # per-image batched w-pair maxes from SBUF into direct quadrants
            # + compact wp staging for the partition-crossing copies
            hmi, wpt = wp_t[b]
            a1b = a1b_t[b]
            hv = hmi[:].rearrange("p h (w two) -> p h w two", two=2)
            nc.vector.tensor_max(out=a1b[0:64, 1:33, 1:65],
                                 in0=hv[0:64, :, :, 0], in1=hv[0:64, :, :, 1])
            nc.vector.tensor_max(out=a1b[64:128, 0:32, 65:129],
                                 in0=hv[64:128, :, :, 0],
                                 in1=hv[64:128, :, :, 1])
            nc.vector.tensor_max(out=wpt[:], in0=hv[:, :, :, 0],
                                 in1=hv[:, :, :, 1])
            # crossing quadrants from compact wp (contiguous src), 2 rings
            nc.sync.dma_start(out=a1b[64:128, 0:32, 1:65], in_=wpt[0:64])
            nc.scalar.dma_start(out=a1b[0:64, 1:33, 65:129], in_=wpt[64:128])

        def conv2_block(b, n):
            a1b = a1b_t[b]
            a2p = a2p_t[b]
            h0 = 4 * n
            pt = ps2.tile([128, 4, 128], F32, tag="p2", bufs=4)
            for kw in range(3):
                nc.tensor.matmul(
                    pt[:], w2p_sb[:, kw, :],
                    a1b[0:128, h0:h0 + 4, kw:kw + 128],
                    start=(kw == 0), stop=False)
            for kw in range(3):
                nc.tensor.matmul(
                    pt[:], w2s_sb[64:128, kw, :],
                    a1b[64:128, h0 + 1:h0 + 5, kw:kw + 128],
                    start=False, stop=(kw == 2))
            s2 = st2.tile([128, 4, 128], F16, tag="s2")
            nc.scalar.activation(
                out=s2[:].rearrange("p a b -> p (a b)"), in_=pt[:],
                func=AF.Identity, bias=b2_sb[:, 0:1], scale=1.0)
            w2m = w2m_pool.tile([128, 4, 64], F16, tag="w2m")
            s2v = s2[:].rearrange("p h (w two) -> p h w two", two=2)
            nc.vector.tensor_max(out=w2m[:], in0=s2v[:, :, :, 0],
                                 in1=s2v[:, :, :, 1])
            wv = w2m[:].rearrange("p (h two) w -> p h two w", two=2)
            nc.vector.tensor_max(out=a2p[:, 1 + 2 * n:3 + 2 * n, 1:65],
                                 in0=wv[:, :, 0, :], in1=wv[:, :, 1, :])

        LEAD = 3
        for b in range(B + LEAD):
            if b < B:
                xsa, xsb = conv1_setup(b)
                for j in range(16):
                    conv1_mm(b, xsa, xsb, j)
                    # interleave conv2 blocks of the image LEAD behind
                    if b >= LEAD and j % 2 == 1:
                        conv2_block(b - LEAD, j // 2)
                conv1_pool(b)
            else:
                for n in range(8):
                    conv2_block(b - LEAD, n)
        # big conv6 weights on the sync ring, after the conv1/2-era DMAs
        nc.sync.dma_start(
            out=w6_sb[:],
            in_=_ap(w6T, 0, [[512, 128], [9 * 128 * 512, 4], [128 * 512, 9],
                             [1, 512]]))

        pclose(wp_pool)
        pclose(w2m_pool)
        pclose(st2)
        pclose(a1b_pool)
        pclose(hmt_pool)
        pclose(xs_pool)
        pclose(ps2)
        pclose(ps1)
        ps34 = popen("ps34", 6, space="PSUM")
        if debug:
            for b in range(B):
                nc.sync.dma_start(
                    out=_ap(dbg["a2"], b * 1024, [[8192, 128], [64, 16],
                                                  [1, 64]]),
                    in_=a2p_t[b][:, 1:17, 1:65])

        # ---- conv3 -> a3_t[g] [128, B, 18, 66] x2 ----
        a3_pool = popen("a3", 2, side="left")
        a3_t = []
        for g in range(2):
            t = a3_pool.tile([128, B, 18, 66], F16, tag="a3", name=f"a3_{g}")
            a3_t.append(t)
            for r in (0, 17):
                nc.scalar.memzero(t[:, :, r, :])
            nc.scalar.memzero(t[:, :, :, 0:2])
            nc.scalar.memzero(t[:, :, :, 64:66])
        for b in range(B):
            for m in range(2):
                for n in range(2):
                    pt = ps34.tile([128, 8, 64], F32, tag="ps")
                    for kh in range(3):
                        for kw in range(3):
                            tap = kh * 3 + kw
                            nc.tensor.matmul(
                                pt[:], w3_sb[:, tap, 128 * m:128 * m + 128],
                                a2p_t[b][:, 8 * n + kh:8 * n + kh + 8,
                                         kw:kw + 64],
                                start=(tap == 0), stop=(tap == 8))
                    nc.scalar.activation(
                        out=a3_t[m][:, b, 1 + 8 * n:9 + 8 * n, 1:65],
                        in_=pt[:], func=AF.Identity, bias=b3_sb[:, m:m + 1],
                        scale=1.0)
        pclose(a2p_pool)

        # ---- conv4 (even rows only) + pool4 -> a4p_t[g] [128, B, 10, 34] ----
        a4p_pool = popen("a4p", 2, side="right")
        a4p_t = []
        for g in range(2):
            t = a4p_pool.tile([128, B, 10, 34], F16, tag="a4p", name=f"a4p{g}")
            a4p_t.append(t)
            for r in (0, 9):
                nc.scalar.memzero(t[:, :, r, :])
            nc.scalar.memzero(t[:, :, :, 0:2])
            nc.scalar.memzero(t[:, :, :, 32:34])
        s4_pool = popen("s4", 3, side="left")
        for b in range(B):
            for m in range(2):
                pt = ps34.tile([128, 8, 64], F32, tag="ps")
                for g in range(2):
                    for kh in range(3):
                        for kw in range(3):
                            i = g * 9 + kh * 3 + kw
                            # even output rows only: rhs h-stride 2
                            nc.tensor.matmul(
                                pt[:], w4_sb[:, g, kh * 3 + kw,
                                             128 * m:128 * m + 128],
                                a3_t[g][:, b, kh:kh + 16:2, kw:kw + 64],
                                start=(i == 0), stop=(i == 17))
                s4 = s4_pool.tile([128, 8, 64], F16, tag="s4")
                nc.scalar.activation(
                    out=s4[:].rearrange("p a b -> p (a b)"), in_=pt[:],
                    func=AF.Identity, bias=b4_sb[:, m:m + 1], scale=1.0)
                s4v = s4[:].rearrange("p h (w two) -> p h w two", two=2)
                nc.vector.tensor_max(
                    out=a4p_t[m][:, b, 1:9, 1:33],
                    in0=s4v[:, :, :, 0], in1=s4v[:, :, :, 1])
        pclose(s4_pool)
        pclose(a3_pool)
        pclose(wA)
        if debug:
            for m in range(2):
                for b in range(B):
                    nc.sync.dma_start(
                        out=_ap(dbg["a4"], m * 2048 + b * 256,
                                [[4096, 128], [32, 8], [1, 32]]),
                        in_=a4p_t[m][:, b, 1:9, 1:33])

        # ---- LSTM weights (whh, lbias, wih) — load during conv5/6 ----
        wE1 = popen("wE1", 1, side="left")
        whh_sb = wE1.tile([128, 2, 2, 1024], F16)
        nc.gpsimd.dma_start(
            out=whh_sb[:],
            in_=_ap(whhT, 0, [[1024, 128], [2 * 128 * 1024, 2],
                              [128 * 1024, 2], [1, 1024]]))
        lb_sb = wE1.tile([128, 2, 8], F32)
        nc.sync.dma_start(out=lb_sb[:], in_=lbias)
        wih_sb = wE1.tile([128, 2, 8, 1024], F16)
        nc.scalar.dma_start(
            out=wih_sb[:, 0],
            in_=_ap(wihT, 0, [[1024, 128], [128 * 1024, 8], [1, 1024]]))
        nc.sync.dma_start(
            out=wih_sb[:, 1],
            in_=_ap(wihT, 8 * 128 * 1024,
                    [[1024, 128], [128 * 1024, 8], [1, 1024]]))

        # ---- conv5 (m-outer) -> c5_t[m]; stats m-pairs -> AllReduce ----
        pclose(ps34)
        ps56 = popen("ps56", 1, space="PSUM")
        stat_pool = popen("stat", 1, side="left")
        scr_pool = popen("scr", 2, side="left")
        c5_pool = popen("c5", 4, side="left")
        c5_t = [c5_pool.tile([128, B, 8, 32], F16, tag="c5", name=f"c5_{m}")
                for m in range(4)]
        # c6p/w7/a5 allocated early: border zeros run during conv4/5
        c6p_pool = popen("c6p", 4, side="right")
        c6p_t = []
        for g in range(4):
            t = c6p_pool.tile([128, B, 4, 17], F16, tag="c6p", name=f"c6p{g}")
            c6p_t.append(t)
            nc.vector.memset(t[:, :, :, 15:17], 0.0)
        wD = popen("wD", 1, side="right")
        w7_sb = wD.tile([128, 4, 4, 512], F16)
        nc.scalar.dma_start(
            out=w7_sb[:],
            in_=_ap(w7T, 0, [[512, 128], [4 * 128 * 512, 4], [128 * 512, 4],
                             [1, 512]]))
        a5_pool = popen("a5", 4, side="right")
        a5_t = []
        for g in range(4):
            t = a5_pool.tile([128, B, 10, 34], F16, tag="a5", name=f"a5_{g}")
            a5_t.append(t)
            for r in (0, 9):
                nc.scalar.memzero(t[:, :, r, :])
            nc.scalar.memzero(t[:, :, :, 0:2])
            nc.scalar.memzero(t[:, :, :, 32:34])
        st5a = stat_pool.tile([128, 4, 4], F32)   # per-m, per-bp sums
        st5b = stat_pool.tile([128, 4, 4], F32)
        cc5i = [dram.tile([128, 2 if h == 0 else 6], F32, tag=f"cc5i{h}",
                          name=f"cc5i{h}") for h in range(2)]
        cc5o = [dram.tile([128, 2 if h == 0 else 6], F32, tag=f"cc5o{h}",
                          name=f"cc5o{h}") for h in range(2)]
        g5 = stat_pool.tile([128, 4, 2], F32)  # per-m (sum, sumsq)
        for m in range(4):
            for bp in range(4):
                b0 = 2 * bp
                pt = ps56.tile([128, 2, 8, 32], F32, tag="p5", bufs=2)
                for g in range(2):
                    for kh in range(3):
                        for kw in range(3):
                            i = g * 9 + kh * 3 + kw
                            nc.tensor.matmul(
                                pt[:], w5_sb[:, g, kh * 3 + kw,
                                             128 * m:128 * m + 128],
                                a4p_t[g][:, b0:b0 + 2, kh:kh + 8, kw:kw + 32],
                                start=(i == 0), stop=(i == 17))
                nc.scalar.activation(
                    out=c5_t[m][:, b0:b0 + 2, :, :], in_=pt[:],
                    func=AF.Identity, bias=b5_sb[:, m:m + 1], scale=1.0,
                    accum_out=st5a[:, m, bp:bp + 1])
                scr = scr_pool.tile([128, 512], F16, tag="scr")
                nc.scalar.activation(
                    out=scr[:], in_=pt[:].rearrange("p a b c -> p (a b c)"),
                    func=AF.Square, bias=b5_sb[:, m:m + 1], scale=1.0,
                    accum_out=st5b[:, m, bp:bp + 1])
            if m == 0:      # AR batch 1: m0 alone, fires earliest
                pk = stat_pool.tile([128, 2], F32, name="pk5a")
                nc.vector.tensor_reduce(out=pk[:, 0:1], in_=st5a[:, 0, :],
                                        axis=AX.X, op=ALU.add)
                nc.vector.tensor_reduce(out=pk[:, 1:2], in_=st5b[:, 0, :],
                                        axis=AX.X, op=ALU.add)
                nc.sync.dma_start(out=cc5i[0][:], in_=pk[:])
                nc.gpsimd.collective_compute(
                    "AllReduce", ALU.add,
                    replica_groups=[list(range(NCORES))],
                    ins=[cc5i[0][:].opt()], outs=[cc5o[0][:].opt()])
                nc.sync.dma_start(out=g5[:, 0], in_=cc5o[0][:])
            elif m == 3:    # AR batch 2: m1..m3
                pk = stat_pool.tile([128, 6], F32, name="pk5b")
                for j, mm in enumerate((1, 2, 3)):
                    nc.vector.tensor_reduce(out=pk[:, 2 * j:2 * j + 1],
                                            in_=st5a[:, mm, :],
                                            axis=AX.X, op=ALU.add)
                    nc.vector.tensor_reduce(out=pk[:, 2 * j + 1:2 * j + 2],
                                            in_=st5b[:, mm, :],
                                            axis=AX.X, op=ALU.add)
                nc.sync.dma_start(out=cc5i[1][:], in_=pk[:])
                nc.gpsimd.collective_compute(
                    "AllReduce", ALU.add,
                    replica_groups=[list(range(NCORES))],
                    ins=[cc5i[1][:].opt()], outs=[cc5o[1][:].opt()])
                nc.sync.dma_start(
                    out=g5[:, 1:4].rearrange("p a b -> p (a b)"),
                    in_=cc5o[1][:])

        def bn_coeffs(gtile, h, aa, dd, pool, tag):
            # gtile[:, h] = [2m, (sum, sumsq)] -> aa/dd [128, 2]
            ms = pool.tile([128, 2, 2], F32, tag=f"{tag}ms", name=f"{tag}ms{h}")
            nc.scalar.activation(
                out=ms[:].rearrange("p a b -> p (a b)"),
                in_=gtile[:, h].rearrange("p a b -> p (a b)"),
                func=AF.Copy, bias=0.0, scale=INV_N)
            var = pool.tile([128, 2], F32, tag=f"{tag}v", name=f"{tag}v{h}")
            nc.vector.tensor_mul(out=var[:], in0=ms[:, :, 0], in1=ms[:, :, 0])
            nc.vector.tensor_sub(out=var[:], in0=ms[:, :, 1], in1=var[:])
            std = pool.tile([128, 2], F32, tag=f"{tag}s", name=f"{tag}s{h}")
            nc.scalar.activation(out=std[:], in_=var[:], func=AF.Sqrt,
                                 bias=eps_sb[:, 0:1], scale=1.0)
            nc.vector.reciprocal(out=std[:], in_=std[:])
            nc.vector.tensor_mul(out=aa[:], in0=std[:],
                                 in1=gam_sb[:, 2 * h:2 * h + 2])
            nc.vector.tensor_mul(out=dd[:], in0=ms[:, :, 0], in1=aa[:])
            nc.vector.tensor_sub(out=dd[:], in0=bet_sb[:, 2 * h:2 * h + 2],
                                 in1=dd[:])

        # ---- BN5 -> a5_t[g] x4 (padded) ----
        def bn_coeff_batch(gsl, gam, bet, nm, pool, tag):
            # gsl: [128, nm, 2] global (sum, sumsq); -> aa, dd [128, nm]
            ms = pool.tile([128, nm, 2], F32, name=f"{tag}ms")
            nc.scalar.activation(
                out=ms[:].rearrange("p a b -> p (a b)"),
                in_=gsl.rearrange("p a b -> p (a b)"),
                func=AF.Copy, bias=0.0, scale=INV_N)
            var = pool.tile([128, nm], F32, name=f"{tag}v")
            nc.vector.tensor_mul(out=var[:], in0=ms[:, :, 0], in1=ms[:, :, 0])
            nc.vector.tensor_sub(out=var[:], in0=ms[:, :, 1], in1=var[:])
            std = pool.tile([128, nm], F32, name=f"{tag}s")
            nc.scalar.activation(out=std[:], in_=var[:], func=AF.Sqrt,
                                 bias=eps_sb[:, 0:1], scale=1.0)
            nc.vector.reciprocal(out=std[:], in_=std[:])
            aa = pool.tile([128, nm], F32, name=f"{tag}aa")
            dd = pool.tile([128, nm], F32, name=f"{tag}dd")
            nc.vector.tensor_mul(out=aa[:], in0=std[:], in1=gam)
            nc.vector.tensor_mul(out=dd[:], in0=ms[:, :, 0], in1=aa[:])
            nc.vector.tensor_sub(out=dd[:], in0=bet, in1=dd[:])
            return aa, dd

        # m0: coeffs+apply gated only on AR1 (lands mid-conv5)
        aa0, dd0 = bn_coeff_batch(g5[:, 0:1], gam_sb[:, 0:1], bet_sb[:, 0:1],
                                  1, stat_pool, "b5m0")
        nc.scalar.activation(
            out=a5_t[0][:, :, 1:9, 1:33], in_=c5_t[0][:],
            func=AF.Identity, bias=dd0[:, 0:1], scale=aa0[:, 0:1])
        # m1..3: batched coeffs after AR2; applies split across engines
        aa3, dd3 = bn_coeff_batch(g5[:, 1:4], gam_sb[:, 1:4], bet_sb[:, 1:4],
                                  3, stat_pool, "b5m123")
        nc.scalar.activation(
            out=a5_t[1][:, 0:4, 1:9, 1:33], in_=c5_t[1][:, 0:4],
            func=AF.Identity, bias=dd3[:, 0:1], scale=aa3[:, 0:1])
        nc.scalar.activation(
            out=a5_t[2][:, :, 1:9, 1:33], in_=c5_t[2][:],
            func=AF.Identity, bias=dd3[:, 1:2], scale=aa3[:, 1:2])
        nc.scalar.activation(
            out=a5_t[1][:, 4:8, 1:9, 1:33], in_=c5_t[1][:, 4:8],
            func=AF.Identity, bias=dd3[:, 0:1], scale=aa3[:, 0:1])
        nc.scalar.activation(
            out=a5_t[3][:, :, 1:9, 1:33], in_=c5_t[3][:],
            func=AF.Identity, bias=dd3[:, 2:3], scale=aa3[:, 2:3])
        pclose(c5_pool)
        if debug:
            for m in range(4):
                for b in range(B):
                    nc.sync.dma_start(
                        out=_ap(dbg["a5"], m * 2048 + b * 256,
                                [[8192, 128], [32, 8], [1, 32]]),
                        in_=a5_t[m][:, b, 1:9, 1:33])

        # ---- conv6: m0/m1 g0 prework covers BN5-AR2; early BN6 m0-2 ----
        pclose(ps56)
        ps6 = popen("ps6", 1, space="PSUM")
        c6_pool = popen("c6", 4, side="left")
        c6_t = [c6_pool.tile([128, B, 8, 32], F16, tag="c6", name=f"c6_{m}")
                for m in range(4)]
        st6a = stat_pool.tile([128, 4, 4], F32)
        st6b = stat_pool.tile([128, 4, 4], F32)
        cc6i = [dram.tile([128, 6 if h == 0 else 2], F32, tag=f"cc6i{h}",
                          name=f"cc6i{h}") for h in range(2)]
        cc6o = [dram.tile([128, 6 if h == 0 else 2], F32, tag=f"cc6o{h}",
                          name=f"cc6o{h}") for h in range(2)]
        g6 = stat_pool.tile([128, 4, 2], F32)
        pts6 = {}

        def c6_alloc(m):
            pts6[m] = [ps6.tile([128, 2, 8, 32], F32, tag="p6", bufs=8,
                                name=f"p6_{m}_{bp}") for bp in range(4)]

        def c6_mm(m, g):
            for tap in range(9):
                kh, kw = tap // 3, tap % 3
                ii = g * 9 + tap
                for bp in range(4):
                    b0 = 2 * bp
                    nc.tensor.matmul(
                        pts6[m][bp][:],
                        w6_sb[:, g, tap, 128 * m:128 * m + 128],
                        a5_t[g][:, b0:b0 + 2, kh:kh + 8, kw:kw + 32],
                        start=(ii == 0), stop=(ii == 35))

        def c6_drain(m):
            for bp in range(4):
                b0 = 2 * bp
                nc.scalar.activation(
                    out=c6_t[m][:, b0:b0 + 2, :, :], in_=pts6[m][bp][:],
                    func=AF.Identity, bias=b6_sb[:, m:m + 1], scale=1.0,
                    accum_out=st6a[:, m, bp:bp + 1])
                scr = scr_pool.tile([128, 512], F16, tag="scr")
                nc.scalar.activation(
                    out=scr[:], in_=pts6[m][bp][:].rearrange(
                        "p a b c -> p (a b c)"),
                    func=AF.Square, bias=b6_sb[:, m:m + 1], scale=1.0,
                    accum_out=st6b[:, m, bp:bp + 1])
            # pool6: w-pairs of even h rows (pre-BN, gamma>0)
            c6v = c6_t[m][:].rearrange("p b (h two) (w v) -> p b h two w v",
                                       two=2, v=2)
            nc.vector.tensor_max(out=c6p_t[m][:, :, :, 0:16],
                                 in0=c6v[:, :, :, 0, :, 0],
                                 in1=c6v[:, :, :, 0, :, 1])

        c6_alloc(0)
        c6_mm(0, 0)
        c6_alloc(1)
        c6_mm(1, 0)
        for g in (1, 2, 3):
            c6_mm(0, g)
        c6_drain(0)
        for g in (1, 2, 3):
            c6_mm(1, g)
        c6_drain(1)
        c6_alloc(2)
        for g in range(4):
            c6_mm(2, g)
        c6_drain(2)
        # AR batch 1: m0..m2
        pk6a = stat_pool.tile([128, 6], F32, name="pk6a")
        for j in range(3):
            nc.vector.tensor_reduce(out=pk6a[:, 2 * j:2 * j + 1],
                                    in_=st6a[:, j, :], axis=AX.X, op=ALU.add)
            nc.vector.tensor_reduce(out=pk6a[:, 2 * j + 1:2 * j + 2],
                                    in_=st6b[:, j, :], axis=AX.X, op=ALU.add)
        nc.sync.dma_start(out=cc6i[0][:], in_=pk6a[:])
        nc.gpsimd.collective_compute(
            "AllReduce", ALU.add,
            replica_groups=[list(range(NCORES))],
            ins=[cc6i[0][:].opt()], outs=[cc6o[0][:].opt()])
        nc.sync.dma_start(
            out=g6[:, 0:3].rearrange("p a b -> p (a b)"), in_=cc6o[0][:])
        # BN6 coeffs+applies for m0-2, emitted before m3 mms (run under them)
        aa6a, dd6a = bn_coeff_batch(g6[:, 0:3], gam_sb[:, 0:3],
                                    bet_sb[:, 0:3], 3, stat_pool, "b6a")
        for mm_ in range(3):
            nc.scalar.activation(
                out=c6p_t[mm_][:], in_=c6p_t[mm_][:], func=AF.Identity,
                bias=dd6a[:, mm_:mm_ + 1], scale=aa6a[:, mm_:mm_ + 1])
        c6_alloc(3)
        for g in range(4):
            c6_mm(3, g)
        c6_drain(3)
        # AR batch 2: m3 alone
        pk6b = stat_pool.tile([128, 2], F32, name="pk6b")
        nc.vector.tensor_reduce(out=pk6b[:, 0:1], in_=st6a[:, 3, :],
                                axis=AX.X, op=ALU.add)
        nc.vector.tensor_reduce(out=pk6b[:, 1:2], in_=st6b[:, 3, :],
                                axis=AX.X, op=ALU.add)
        nc.sync.dma_start(out=cc6i[1][:], in_=pk6b[:])
        nc.gpsimd.collective_compute(
            "AllReduce", ALU.add,
            replica_groups=[list(range(NCORES))],
            ins=[cc6i[1][:].opt()], outs=[cc6o[1][:].opt()])
        nc.sync.dma_start(out=g6[:, 3], in_=cc6o[1][:])
        pclose(a5_pool)
        pclose(c6_pool)
        aa6b, dd6b = bn_coeff_batch(g6[:, 3:4], gam_sb[:, 3:4],
                                    bet_sb[:, 3:4], 1, stat_pool, "b6b")
        nc.scalar.activation(
            out=c6p_t[3][:], in_=c6p_t[3][:], func=AF.Identity,
            bias=dd6b[:, 0:1], scale=aa6b[:, 0:1])
        pclose(scr_pool)
        if debug:
            for g in range(4):
                for b in range(B):
                    nc.sync.dma_start(
                        out=_ap(dbg["c6p"], g * 512 + b * 64,
                                [[2048, 128], [16, 4], [1, 16]]),
                        in_=c6p_t[g][:, b, :, 0:16])

        # ---- conv7 (VALID 2x2, g-outer) -> c7 [128, 4, B, 3, 16] ----
        pclose(ps6)
        ps7 = popen("ps7", 4, space="PSUM")
        c7_pool = popen("c7", 1, side="left")
        c7 = c7_pool.tile([128, 4, B, 3, 16], F16)
        pts7 = [ps7.tile([128, 8, 3, 16], F32, tag="p7", name=f"p7_{m}")
                for m in range(4)]
        for g in range(4):
            for tap in range(4):
                kh, kw = tap // 2, tap % 2
                i = g * 4 + tap
                for m in range(4):
                    nc.tensor.matmul(
                        pts7[m][:], w7_sb[:, g, tap, 128 * m:128 * m + 128],
                        c6p_t[g][:, :, kh:kh + 3, kw:kw + 16],
                        start=(i == 0), stop=(i == 15))
        for m in range(4):
            nc.scalar.activation(out=c7[:, m], in_=pts7[m][:],
                                 func=AF.Identity, bias=b7_sb[:, m:m + 1],
                                 scale=1.0)
        pclose(wD)
        pclose(c6p_pool)
        if debug:
            nc.sync.dma_start(
                out=_ap(dbg["c7"], 0, [[1536, 128], [1, 1536]]),
                in_=c7[:].rearrange("p g b h w -> p (g b h w)"))

        # ---- LSTM ----
        pclose(ps7)
        psL = popen("psL", 1, space="PSUM")
        ls = popen("ls", 1, side="right")
        xg = ls.tile([128, 2, 8, 8, 15], F32)
        hs_d = [ls.tile([128, 2, 8, 15], F16, name=f"hs{dr}")
                for dr in range(2)]
        cst_d = [ls.tile([128, 2, 8], F32, name=f"cst{dr}") for dr in range(2)]
        gp = popen("gp", 4, side="right")
        tp = popen("tp", 6, side="right")

        for dr in range(2):
            for m in range(8):
                pt = psL.tile([128, 8, 16], F32, tag="px", bufs=2)
                for gd in range(8):
                    nc.tensor.matmul(
                        pt[:], wih_sb[:, dr, gd, 128 * m:128 * m + 128],
                        c7[:, gd % 4, :, gd // 4, :],
                        start=(gd == 0), stop=(gd == 7))
                nc.scalar.activation(
                    out=xg[:, dr, m],
                    in_=pt[:, :, 0:15],
                    func=AF.Identity, bias=lb_sb[:, dr, m:m + 1], scale=1.0)
        if debug:
            nc.sync.dma_start(
                out=_ap(dbg["xg"], 0, [[1920, 128], [1, 1920]]),
                in_=xg[:].rearrange("p d m b t -> p (d m b t)"))

        for t in range(15):
            gas = {}
            for dr in range(2):
                tt = t if dr == 0 else 14 - t
                ga = gp.tile([128, 8, 8], F32, tag="ga", name=f"ga{dr}")
                gas[dr] = ga
                if t == 0:
                    nc.vector.tensor_copy(out=ga[:], in_=xg[:, dr, :, :, tt])
                else:
                    tprev = tt - 1 if dr == 0 else tt + 1
                    pr = psL.tile([128, 8, 8], F32, tag=f"pr{dr}", bufs=2,
                                  name=f"pr{dr}")
                    for m in range(8):
                        for gh in range(2):
                            nc.tensor.matmul(
                                pr[:, m, :],
                                whh_sb[:, dr, gh, 128 * m:128 * m + 128],
                                hs_d[dr][:, gh, :, tprev],
                                start=(gh == 0), stop=(gh == 1),
                                skip_group_check=True)
                    nc.vector.tensor_add(out=ga[:], in0=pr[:],
                                         in1=xg[:, dr, :, :, tt])
            for dr in range(2):
                tt = t if dr == 0 else 14 - t
                ga = gas[dr]
                nc.scalar.activation(out=ga[:, 0:6, :], in_=ga[:, 0:6, :],
                                     func=AF.Sigmoid, bias=0.0, scale=1.0)
                nc.scalar.activation(out=ga[:, 6:8, :], in_=ga[:, 6:8, :],
                                     func=AF.Tanh, bias=0.0, scale=1.0)
                cs = cst_d[dr][:]
                if t == 0:
                    nc.vector.tensor_mul(out=cs, in0=ga[:, 0:2, :],
                                         in1=ga[:, 6:8, :])
                else:
                    t2 = tp.tile([128, 2, 8], F32, tag="t2", name=f"t2_{dr}")
                    nc.vector.tensor_mul(out=t2[:], in0=ga[:, 2:4, :], in1=cs)
                    t1 = tp.tile([128, 2, 8], F32, tag="t1", name=f"t1_{dr}")
                    nc.vector.tensor_mul(out=t1[:], in0=ga[:, 0:2, :],
                                         in1=ga[:, 6:8, :])
                    nc.vector.tensor_add(out=cs, in0=t1[:], in1=t2[:])
                th = tp.tile([128, 2, 8], F32, tag="th", name=f"th_{dr}")
                nc.scalar.activation(out=th[:], in_=cs, func=AF.Tanh,
                                     bias=0.0, scale=1.0)
                nc.vector.tensor_mul(out=hs_d[dr][:, :, :, tt],
                                     in0=ga[:, 4:6, :], in1=th[:])

        if debug:
            for dr in range(2):
                nc.sync.dma_start(
                    out=_ap(dbg["hs"], dr * 240, [[480, 128], [1, 240]]),
                    in_=hs_d[dr][:].rearrange("p g b t -> p (g b t)"))

        # ---- output: PE-transpose h to (b*t)-partitions, contiguous DMA ----
        ptt = psL.tile([120, 512], F16, tag="pt_out")
        for dr in range(2):
            for gh in range(2):
                nc.tensor.transpose(
                    ptt[:, 128 * (2 * dr + gh):128 * (2 * dr + gh) + 128],
                    hs_d[dr][:, gh].rearrange("p b t -> p (b t)"),
                    ident16[:])
        outsb = ls.tile([120, 512], F16)
        nc.scalar.copy(out=outsb[:], in_=ptt[:])
        nc.sync.dma_start(out=_ap(out, 0, [[512, 60], [1, 512]]),
                          in_=outsb[0:60])
        nc.scalar.dma_start(out=_ap(out, 60 * 512, [[512, 60], [1, 512]]),
                            in_=outsb[60:120])

        for p in reversed(list(opened)):
            pclose(p)

    nc.compile()
    return nc


def prep_inputs(inputs, core):
    """Host-side: shard + transform weights for one core."""
    d = {}
    x = np.asarray(inputs["x"], dtype=np.float32)
    xs = x[core * B:(core + 1) * B, 0]          # (8, 64, 256)
    xp = np.zeros((B, 66, 258), np.float32)
    xp[:, 1:65, 1:257] = xs
    # xim19: c<9: tap c at x; 9<=c<18: tap c-9 at x+128; c=18: ones
    xim = np.empty((B, 19, 64, 128), np.float32)
    for kh in range(3):
        for kw in range(3):
            xim[:, kh * 3 + kw] = xp[:, kh:kh + 64, kw:kw + 128]
            xim[:, 9 + kh * 3 + kw] = xp[:, kh:kh + 64, 128 + kw:256 + kw]
    xim[:, 18] = 1.0
    d["xim"] = xim.astype(np.float16)

    w1 = np.asarray(inputs["w1"], np.float32)   # (64,1,3,3)
    w1f = w1[:, 0].reshape(64, 9).T             # (9, 64)
    b1 = np.asarray(inputs["b1"], np.float32)
    w1x = np.zeros((19, 128), np.float32)
    w1x[0:9, 0:64] = w1f
    w1x[9:18, 64:128] = w1f
    w1x[18, 0:64] = b1
    w1x[18, 64:128] = b1
    d["w1x"] = w1x.astype(np.float16)

    w2 = np.asarray(inputs["w2"], np.float32)   # (128,64,3,3)
    w2p = np.zeros((3, 128, 128), np.float32)
    for kw in range(3):
        w2p[kw, 0:64] = w2[:, :, 0, kw].T
        w2p[kw, 64:128] = w2[:, :, 1, kw].T
    d["w2p"] = w2p.astype(np.float16)
    d["w2s"] = np.ascontiguousarray(
        np.transpose(w2[:, :, 2, :], (2, 1, 0))).astype(np.float16)  # (3, 64, 128)
    d["b2"] = np.asarray(inputs["b2"], np.float32).reshape(1, 128).T.copy()

    def wT(w, gK, cout):
        o, i_, kh, kw = w.shape
        r = np.transpose(w, (2, 3, 1, 0)).reshape(kh * kw, gK, 128, o)
        return np.ascontiguousarray(np.transpose(r, (1, 0, 2, 3))).astype(np.float16)

    d["w3T"] = wT(np.asarray(inputs["w3"], np.float32), 1, 256)
    d["w4T"] = wT(np.asarray(inputs["w4"], np.float32), 2, 256)
    d["w5T"] = wT(np.asarray(inputs["w5"], np.float32), 2, 512)
    d["w6T"] = wT(np.asarray(inputs["w6"], np.float32), 4, 512)
    w7 = np.asarray(inputs["w7"], np.float32)   # (512,512,2,2)
    r7 = np.transpose(w7, (2, 3, 1, 0)).reshape(4, 4, 128, 512)
    d["w7T"] = np.ascontiguousarray(np.transpose(r7, (1, 0, 2, 3))).astype(np.float16)
    for k, g in (("b3", 2), ("b4", 2), ("b5", 4), ("b6", 4), ("b7", 4)):
        d[k] = np.ascontiguousarray(
            np.asarray(inputs[k], np.float32).reshape(g, 128).T)
    d["gam"] = np.ascontiguousarray(
        np.asarray(inputs["gamma"], np.float32).reshape(4, 128).T)
    d["bet"] = np.ascontiguousarray(
        np.asarray(inputs["beta"], np.float32).reshape(4, 128).T)

    # LSTM: d-column permutation dmap maps compute-chunk col 128*j+p to
    # reference D index 2*(128*(j%4)+p) + j//4
    j = np.arange(8)[:, None]
    p = np.arange(128)[None, :]
    dmap = (2 * (128 * (j % 4) + p) + j // 4).reshape(-1)
    wih = np.stack([np.asarray(inputs["Wih_f"], np.float32),
                    np.asarray(inputs["Wih_b"], np.float32)])
    whh = np.stack([np.asarray(inputs["Whh_f"], np.float32),
                    np.asarray(inputs["Whh_b"], np.float32)])
    wihp = wih[:, PERM4H][:, :, dmap]           # (2, 1024, 1024)
    d["wihT"] = np.ascontiguousarray(
        np.transpose(wihp, (0, 2, 1)).reshape(2, 8, 128, 1024)).astype(np.float16)
    whhp = whh[:, PERM4H]                       # (2, 1024, 256)
    d["whhT"] = np.ascontiguousarray(
        np.transpose(whhp, (0, 2, 1)).reshape(2, 2, 128, 1024)).astype(
            np.float16)
    lb = (np.stack([np.asarray(inputs["bih_f"], np.float32),
                    np.asarray(inputs["bih_b"], np.float32)])
          + np.stack([np.asarray(inputs["bhh_f"], np.float32),
                      np.asarray(inputs["bhh_b"], np.float32)]))
    lbp = lb[:, PERM4H].reshape(2, 8, 128)      # (dir, m, p)
    d["lbias"] = np.ascontiguousarray(np.transpose(lbp, (2, 0, 1)))
    return d


_NC_CACHE = {}


def kernel(**inputs):
    key = "debug" if inputs.pop("_debug", False) else "main"
    if key not in _NC_CACHE:
        _NC_CACHE[key] = build(debug=(key == "debug"))
    nc = _NC_CACHE[key]
    in_maps = [prep_inputs(inputs, c) for c in range(NCORES)]
    res = bass_utils.run_bass_kernel_spmd(nc, in_maps,
                                          core_ids=list(range(NCORES)))
    out = np.concatenate(
        [np.asarray(res.results[c]["out"]).astype(np.float32)
         for c in range(NCORES)], axis=0)
    kernel.last_results = res
    return out



# revision 48
# speedup vs baseline: 1.0252x; 1.0252x over previous
"""CaptchaCRNN Trainium2 kernel: 7 convs + 2 train-mode BN + maxpools + biLSTM.

Data-parallel over batch on 8 NeuronCores (8 images/core). BN batch stats are
globalized with pipelined AllReduces overlapped under the next conv. Convs run
in bf16 (1 cyc/row, FWL weight loads); pools drain PSUM via DVE/GPSIMD so the
scalar engine never bottlenecks; conv4 computes only the even rows that
survive its (1,2)x(2,2) maxpool.
"""
import sys

sys.path.insert(0, "/opt/trn_rl_repo")

import numpy as np
import ml_dtypes
import concourse.bass as bass
import concourse.bacc as bacc
import concourse.tile as tile
from concourse import masks
from concourse import mybir
from concourse import bass_utils

F32 = mybir.dt.float32
F16 = mybir.dt.float16
BF16 = mybir.dt.bfloat16
AF = mybir.ActivationFunctionType
ALU = mybir.AluOpType
AX = mybir.AxisListType

NCORES = 8
B = 8          # images per core
EPS = 1e-5
INV_N = 1.0 / (64 * 8 * 32)   # BN normalizer: full batch 64 x H8 x W32

# 4H gate permutation: torch order [i,f,g,o] -> compute order [i,f,o,g]
PERM4H = np.r_[0:512, 768:1024, 512:768]



def _ap(obj, offset, dims):
    base = obj if isinstance(obj, bass.AP) else obj[:]
    return bass.AP(tensor=base.tensor, offset=base.offset + offset,
                   ap=[list(d) for d in dims])


def build(debug=False):
    nc = bacc.Bacc("TRN2", target_bir_lowering=False, debug=False,
                   enable_asserts=True, num_devices=NCORES)

    def din(name, shape, dt=F32):
        return nc.dram_tensor(name, list(shape), dt, kind="ExternalInput").ap()

    def dout(name, shape, dt=F32):
        return nc.dram_tensor(name, list(shape), dt, kind="ExternalOutput").ap()

    xim = din("xim", (B, 19, 64, 128), F16)   # im2col taps, w-half blocked
    w1x = din("w1x", (19, 128), F16)
    w2p = din("w2p", (3, 128, 128), F16)
    w2s = din("w2s", (3, 64, 128), F16)
    w3T = din("w3T", (1, 9, 128, 256), F16)
    w4T = din("w4T", (2, 9, 128, 256), F16)
    w5T = din("w5T", (2, 9, 128, 512), F16)
    w6T = din("w6T", (4, 9, 128, 512), F16)
    w7T = din("w7T", (4, 4, 128, 512), F16)
    b2 = din("b2", (128, 1))
    b3 = din("b3", (128, 2))
    b4 = din("b4", (128, 2))
    b5 = din("b5", (128, 4))
    b6 = din("b6", (128, 4))
    b7 = din("b7", (128, 4))
    gam = din("gam", (128, 4))
    bet = din("bet", (128, 4))
    wihT = din("wihT", (2, 8, 128, 1024), F16)
    whhT = din("whhT", (2, 2, 128, 1024), F16)
    lbias = din("lbias", (128, 2, 8))
    out = dout("out", (B, 15, 512), F16)

    dbg = {}
    if debug:
        dbg["a2"] = dout("dbg_a2", (128, 8, 16, 64), F16)
        dbg["a4"] = dout("dbg_a4", (128, 2, 8, 8, 32), F16)
        dbg["a5"] = dout("dbg_a5", (128, 4, 8, 8, 32), F16)
        dbg["c6p"] = dout("dbg_c6p", (128, 4, 8, 4, 16), F16)
        dbg["c7"] = dout("dbg_c7", (128, 4, 8, 3, 16), F16)
        dbg["xg"] = dout("dbg_xg", (128, 2, 8, 8, 15))
        dbg["hs"] = dout("dbg_hs", (128, 2, 2, 8, 15), F16)

    with tile.TileContext(nc) as tc:
        opened = []

        def popen(name, bufs, space="SBUF", side=None):
            cm = tc.tile_pool(name=name, bufs=bufs, space=space, side=side)
            p = cm.__enter__()
            p._cm = cm
            opened.append(p)
            return p

        def pclose(p):
            p._cm.__exit__(None, None, None)
            opened.remove(p)

        const = popen("const", 1, side="left")
        dram = popen("dram", 1, space="DRAM")

        # ---- constants ----
        b2_sb = const.tile([128, 1], F32)
        nc.sync.dma_start(out=b2_sb[:], in_=b2)
        b3_sb = const.tile([128, 2], F32)
        nc.sync.dma_start(out=b3_sb[:], in_=b3)
        b4_sb = const.tile([128, 2], F32)
        nc.sync.dma_start(out=b4_sb[:], in_=b4)
        b5_sb = const.tile([128, 4], F32)
        nc.sync.dma_start(out=b5_sb[:], in_=b5)
        b6_sb = const.tile([128, 4], F32)
        nc.sync.dma_start(out=b6_sb[:], in_=b6)
        b7_sb = const.tile([128, 4], F32)
        nc.sync.dma_start(out=b7_sb[:], in_=b7)
        gam_sb = const.tile([128, 4], F32)
        nc.sync.dma_start(out=gam_sb[:], in_=gam)
        bet_sb = const.tile([128, 4], F32)
        nc.sync.dma_start(out=bet_sb[:], in_=bet)
        eps_sb = const.tile([128, 1], F32)
        nc.vector.memset(eps_sb[:], EPS)
        ident16 = const.tile([128, 128], F16)
        masks.make_identity(nc, ident16[:])

        # ---- conv1+2 weights ----
        wA = popen("wA", 1, side="left")
        w1_sb = wA.tile([19, 128], F16)
        nc.sync.dma_start(out=w1_sb[:], in_=w1x)
        w2p_sb = wA.tile([128, 3, 128], F16)
        nc.gpsimd.dma_start(
            out=w2p_sb[:],
            in_=_ap(w2p, 0, [[128, 128], [128 * 128, 3], [1, 128]]))
        w2s_sb = wA.tile([128, 3, 128], F16)
        nc.gpsimd.dma_start(
            out=w2s_sb[64:128, :, :],
            in_=_ap(w2s, 0, [[128, 64], [64 * 128, 3], [1, 128]]))
        w3_sb = wA.tile([128, 9, 256], F16)
        nc.gpsimd.dma_start(
            out=w3_sb[:],
            in_=_ap(w3T, 0, [[256, 128], [128 * 256, 9], [1, 256]]))
        w4_sb = wA.tile([128, 2, 9, 256], F16)
        nc.gpsimd.dma_start(
            out=w4_sb[:],
            in_=_ap(w4T, 0, [[256, 128], [9 * 128 * 256, 2], [128 * 256, 9],
                             [1, 256]]))

        # ---- conv1 + a1b assembly + conv2: interleaved, multi-ring DMA ----
        # conv1 psum per mm [128, 4h, 128w]; p<64: ch at w<128, p>=64: w>=128
        xs_pool = popen("xs", 2, side="left")
        hmt_pool = popen("hmt", 4, side="left")
        ps1 = popen("ps1", 4, space="PSUM")
        ps2 = popen("ps2", 4, space="PSUM")
        a1b_pool = popen("a1b", 4, side="left")
        st2 = popen("st2", 3, side="left")
        w2m_pool = popen("w2m", 3, side="left")
        # prefetch w5, w6 during conv1-4 (gpsimd ring)
        wB = popen("wB", 1, side="right")
        w5_sb = wB.tile([128, 2, 9, 512], F16)
        nc.gpsimd.dma_start(
            out=w5_sb[:],
            in_=_ap(w5T, 0, [[512, 128], [9 * 128 * 512, 2], [128 * 512, 9],
                             [1, 512]]))
        w6_sb = wB.tile([128, 4, 9, 512], F16)   # DMA issued after conv1/2
        a2p_pool = popen("a2p", 8, side="right")
        a2p_t = [a2p_pool.tile([128, 18, 66], F16, tag="a2p", name=f"a2p{b}")
                 for b in range(B)]
        # border zeros once, on scalar (idle during conv1); columns are zeroed
        # 2-wide (interior neighbor is rewritten by later compute/DMA)
        for b in range(B):
            for r in (0, 17):
                nc.scalar.memzero(a2p_t[b][:, r, :])
            nc.scalar.memzero(a2p_t[b][:, :, 0:2])
            nc.scalar.memzero(a2p_t[b][:, :, 64:66])

        a1b_t = []   # ring of 4; borders zeroed once, interior rewritten
        wp_pool = popen("wp", 1, side="left")
        wp_t = []

        def conv1_setup(b):
            xsa = xs_pool.tile([19, 32, 128], F16, tag="xsa", bufs=2)
            xsb = xs_pool.tile([19, 32, 128], F16, tag="xsb", bufs=2)
            base = b * (19 * 64 * 128)
            nc.sync.dma_start(
                out=xsa[:], in_=_ap(xim, base, [[64 * 128, 19], [1, 32 * 128]]))
            nc.sync.dma_start(
                out=xsb[:], in_=_ap(xim, base + 32 * 128,
                                    [[64 * 128, 19], [1, 32 * 128]]))
            if b < 4:
                a1b = a1b_pool.tile([128, 34, 130], F16, tag="a1b", bufs=4,
                                    name=f"a1b{b % 4}")
                a1b_t.append(a1b)
                for r in (0, 33):
                    nc.scalar.memzero(a1b[0:64, r, :])
                nc.scalar.memzero(a1b[64:128, 32:34, :])
                nc.scalar.memzero(a1b[:, :, 0:2])
                nc.scalar.memzero(a1b[:, :, 128:130])
            else:
                a1b_t.append(a1b_t[b - 4])
            hmi = wp_pool.tile([128, 32, 128], F16, tag="hmi", bufs=2,
                               name=f"hmi{b % 2}")
            wpt = wp_pool.tile([128, 32, 64], F16, tag="wp", bufs=3,
                               name=f"wp{b % 3}")
            wp_t.append((hmi, wpt))
            return xsa, xsb

        def conv1_mm(b, xsa, xsb, j):
            # one mm: rows 4j..4j+3; h-pair max -> hmi[:, 2j:2j+2, :]
            xs = xsa if j < 8 else xsb
            r0 = 4 * (j % 8)
            hmi = wp_t[b][0]
            pt = ps1.tile([128, 4, 128], F32, tag="p1", bufs=4)
            nc.tensor.matmul(pt[:], w1_sb[:], xs[:, r0:r0 + 4, :],
                             start=True, stop=True)
            s0 = hmt_pool.tile([128, 2, 128], F16, tag="s0", bufs=4)
            nc.scalar.activation(
                out=s0[:].rearrange("p a b -> p (a b)"), in_=pt[:, 0::2, :],
                func=AF.Identity, bias=0.0, scale=1.0)
            nc.vector.tensor_max(out=hmi[:, 2 * j:2 * j + 2, :],
                                 in0=pt[:, 1::2, :], in1=s0[:])

        def conv1_pool(b):
            # per-image batched w-pair maxes from SBUF into direct quadrants
            # + compact wp staging for the partition-crossing copies
            hmi, wpt = wp_t[b]
            a1b = a1b_t[b]
            hv = hmi[:].rearrange("p h (w two) -> p h w two", two=2)
            nc.vector.tensor_max(out=a1b[0:64, 1:33, 1:65],
                                 in0=hv[0:64, :, :, 0], in1=hv[0:64, :, :, 1])
            nc.vector.tensor_max(out=a1b[64:128, 0:32, 65:129],
                                 in0=hv[64:128, :, :, 0],
                                 in1=hv[64:128, :, :, 1])
            nc.vector.tensor_max(out=wpt[:], in0=hv[:, :, :, 0],
                                 in1=hv[:, :, :, 1])
            # crossing quadrants from compact wp on the scalar ring only, so
            # their pool-wait never blocks the sync ring's input loads
            nc.scalar.dma_start(out=a1b[64:128, 0:32, 1:65], in_=wpt[0:64])
            nc.scalar.dma_start(out=a1b[0:64, 1:33, 65:129], in_=wpt[64:128])

        def conv2_block(b, n):
            a1b = a1b_t[b]
            a2p = a2p_t[b]
            h0 = 4 * n
            pt = ps2.tile([128, 4, 128], F32, tag="p2", bufs=4)
            for kw in range(3):
                nc.tensor.matmul(
                    pt[:], w2p_sb[:, kw, :],
                    a1b[0:128, h0:h0 + 4, kw:kw + 128],
                    start=(kw == 0), stop=False)
            for kw in range(3):
                nc.tensor.matmul(
                    pt[:], w2s_sb[64:128, kw, :],
                    a1b[64:128, h0 + 1:h0 + 5, kw:kw + 128],
                    start=False, stop=(kw == 2))
            s2 = st2.tile([128, 4, 128], F16, tag="s2")
            nc.scalar.activation(
                out=s2[:].rearrange("p a b -> p (a b)"), in_=pt[:],
                func=AF.Identity, bias=b2_sb[:, 0:1], scale=1.0)
            w2m = w2m_pool.tile([128, 4, 64], F16, tag="w2m")
            s2v = s2[:].rearrange("p h (w two) -> p h w two", two=2)
            nc.vector.tensor_max(out=w2m[:], in0=s2v[:, :, :, 0],
                                 in1=s2v[:, :, :, 1])
            wv = w2m[:].rearrange("p (h two) w -> p h two w", two=2)
            nc.vector.tensor_max(out=a2p[:, 1 + 2 * n:3 + 2 * n, 1:65],
                                 in0=wv[:, :, 0, :], in1=wv[:, :, 1, :])

        LEAD = 3
        for b in range(B + LEAD):
            if b < B:
                xsa, xsb = conv1_setup(b)
                for j in range(16):
                    conv1_mm(b, xsa, xsb, j)
                    # interleave conv2 blocks of the image LEAD behind
                    if b >= LEAD and j % 2 == 1:
                        conv2_block(b - LEAD, j // 2)
                conv1_pool(b)
            else:
                for n in range(8):
                    conv2_block(b - LEAD, n)
        # big conv6 weights on the sync ring, after the conv1/2-era DMAs
        nc.sync.dma_start(
            out=w6_sb[:],
            in_=_ap(w6T, 0, [[512, 128], [9 * 128 * 512, 4], [128 * 512, 9],
                             [1, 512]]))

        pclose(wp_pool)
        pclose(w2m_pool)
        pclose(st2)
        pclose(a1b_pool)
        pclose(hmt_pool)
        pclose(xs_pool)
        pclose(ps2)
        pclose(ps1)
        ps34 = popen("ps34", 6, space="PSUM")
        if debug:
            for b in range(B):
                nc.sync.dma_start(
                    out=_ap(dbg["a2"], b * 1024, [[8192, 128], [64, 16],
                                                  [1, 64]]),
                    in_=a2p_t[b][:, 1:17, 1:65])

        # ---- conv3 -> a3_t[g] [128, B, 18, 66] x2 ----
        a3_pool = popen("a3", 2, side="left")
        a3_t = []
        for g in range(2):
            t = a3_pool.tile([128, B, 18, 66], F16, tag="a3", name=f"a3_{g}")
            a3_t.append(t)
            for r in (0, 17):
                nc.scalar.memzero(t[:, :, r, :])
            nc.scalar.memzero(t[:, :, :, 0:2])
            nc.scalar.memzero(t[:, :, :, 64:66])
        for b in range(B):
            for m in range(2):
                for n in range(2):
                    pt = ps34.tile([128, 8, 64], F32, tag="ps")
                    for kh in range(3):
                        for kw in range(3):
                            tap = kh * 3 + kw
                            nc.tensor.matmul(
                                pt[:], w3_sb[:, tap, 128 * m:128 * m + 128],
                                a2p_t[b][:, 8 * n + kh:8 * n + kh + 8,
                                         kw:kw + 64],
                                start=(tap == 0), stop=(tap == 8))
                    nc.scalar.activation(
                        out=a3_t[m][:, b, 1 + 8 * n:9 + 8 * n, 1:65],
                        in_=pt[:], func=AF.Identity, bias=b3_sb[:, m:m + 1],
                        scale=1.0)
        pclose(a2p_pool)

        # ---- conv4 (even rows only) + pool4 -> a4p_t[g] [128, B, 10, 34] ----
        a4p_pool = popen("a4p", 2, side="right")
        a4p_t = []
        for g in range(2):
            t = a4p_pool.tile([128, B, 10, 34], F16, tag="a4p", name=f"a4p{g}")
            a4p_t.append(t)
            for r in (0, 9):
                nc.scalar.memzero(t[:, :, r, :])
            nc.scalar.memzero(t[:, :, :, 0:2])
            nc.scalar.memzero(t[:, :, :, 32:34])
        s4_pool = popen("s4", 3, side="left")
        for b in range(B):
            for m in range(2):
                pt = ps34.tile([128, 8, 64], F32, tag="ps")
                for g in range(2):
                    for kh in range(3):
                        for kw in range(3):
                            i = g * 9 + kh * 3 + kw
                            # even output rows only: rhs h-stride 2
                            nc.tensor.matmul(
                                pt[:], w4_sb[:, g, kh * 3 + kw,
                                             128 * m:128 * m + 128],
                                a3_t[g][:, b, kh:kh + 16:2, kw:kw + 64],
                                start=(i == 0), stop=(i == 17))
                s4 = s4_pool.tile([128, 8, 64], F16, tag="s4")
                nc.scalar.activation(
                    out=s4[:].rearrange("p a b -> p (a b)"), in_=pt[:],
                    func=AF.Identity, bias=b4_sb[:, m:m + 1], scale=1.0)
                s4v = s4[:].rearrange("p h (w two) -> p h w two", two=2)
                nc.vector.tensor_max(
                    out=a4p_t[m][:, b, 1:9, 1:33],
                    in0=s4v[:, :, :, 0], in1=s4v[:, :, :, 1])
        pclose(s4_pool)
        pclose(a3_pool)
        pclose(wA)
        if debug:
            for m in range(2):
                for b in range(B):
                    nc.sync.dma_start(
                        out=_ap(dbg["a4"], m * 2048 + b * 256,
                                [[4096, 128], [32, 8], [1, 32]]),
                        in_=a4p_t[m][:, b, 1:9, 1:33])

        # ---- LSTM weights (whh, lbias, wih) — load during conv5/6 ----
        wE1 = popen("wE1", 1, side="left")
        whh_sb = wE1.tile([128, 2, 2, 1024], F16)
        nc.gpsimd.dma_start(
            out=whh_sb[:],
            in_=_ap(whhT, 0, [[1024, 128], [2 * 128 * 1024, 2],
                              [128 * 1024, 2], [1, 1024]]))
        lb_sb = wE1.tile([128, 2, 8], F32)
        nc.sync.dma_start(out=lb_sb[:], in_=lbias)
        wih_sb = wE1.tile([128, 2, 8, 1024], F16)
        nc.scalar.dma_start(
            out=wih_sb[:, 0],
            in_=_ap(wihT, 0, [[1024, 128], [128 * 1024, 8], [1, 1024]]))
        nc.sync.dma_start(
            out=wih_sb[:, 1],
            in_=_ap(wihT, 8 * 128 * 1024,
                    [[1024, 128], [128 * 1024, 8], [1, 1024]]))

        # ---- conv5 (m-outer) -> c5_t[m]; stats m-pairs -> AllReduce ----
        pclose(ps34)
        ps56 = popen("ps56", 1, space="PSUM")
        stat_pool = popen("stat", 1, side="left")
        scr_pool = popen("scr", 2, side="left")
        c5_pool = popen("c5", 4, side="left")
        c5_t = [c5_pool.tile([128, B, 8, 32], F16, tag="c5", name=f"c5_{m}")
                for m in range(4)]
        # c6p/w7/a5 allocated early: border zeros run during conv4/5
        c6p_pool = popen("c6p", 4, side="right")
        c6p_t = []
        for g in range(4):
            t = c6p_pool.tile([128, B, 4, 17], F16, tag="c6p", name=f"c6p{g}")
            c6p_t.append(t)
            nc.vector.memset(t[:, :, :, 15:17], 0.0)
        wD = popen("wD", 1, side="right")
        w7_sb = wD.tile([128, 4, 4, 512], F16)
        nc.scalar.dma_start(
            out=w7_sb[:],
            in_=_ap(w7T, 0, [[512, 128], [4 * 128 * 512, 4], [128 * 512, 4],
                             [1, 512]]))
        a5_pool = popen("a5", 4, side="right")
        a5_t = []
        for g in range(4):
            t = a5_pool.tile([128, B, 10, 34], F16, tag="a5", name=f"a5_{g}")
            a5_t.append(t)
            for r in (0, 9):
                nc.scalar.memzero(t[:, :, r, :])
            nc.scalar.memzero(t[:, :, :, 0:2])
            nc.scalar.memzero(t[:, :, :, 32:34])
        st5a = stat_pool.tile([128, 4, 4], F32)   # per-m, per-bp sums
        st5b = stat_pool.tile([128, 4, 4], F32)
        cc5i = [dram.tile([128, 2 if h == 0 else 6], F32, tag=f"cc5i{h}",
                          name=f"cc5i{h}") for h in range(2)]
        cc5o = [dram.tile([128, 2 if h == 0 else 6], F32, tag=f"cc5o{h}",
                          name=f"cc5o{h}") for h in range(2)]
        g5 = stat_pool.tile([128, 4, 2], F32)  # per-m (sum, sumsq)
        for m in range(4):
            for bp in range(4):
                b0 = 2 * bp
                pt = ps56.tile([128, 2, 8, 32], F32, tag="p5", bufs=2)
                for g in range(2):
                    for kh in range(3):
                        for kw in range(3):
                            i = g * 9 + kh * 3 + kw
                            nc.tensor.matmul(
                                pt[:], w5_sb[:, g, kh * 3 + kw,
                                             128 * m:128 * m + 128],
                                a4p_t[g][:, b0:b0 + 2, kh:kh + 8, kw:kw + 32],
                                start=(i == 0), stop=(i == 17))
                nc.scalar.activation(
                    out=c5_t[m][:, b0:b0 + 2, :, :], in_=pt[:],
                    func=AF.Identity, bias=b5_sb[:, m:m + 1], scale=1.0,
                    accum_out=st5a[:, m, bp:bp + 1])
                scr = scr_pool.tile([128, 512], F16, tag="scr")
                nc.scalar.activation(
                    out=scr[:], in_=pt[:].rearrange("p a b c -> p (a b c)"),
                    func=AF.Square, bias=b5_sb[:, m:m + 1], scale=1.0,
                    accum_out=st5b[:, m, bp:bp + 1])
            if m == 0:      # AR batch 1: m0 alone, fires earliest
                pk = stat_pool.tile([128, 2], F32, name="pk5a")
                nc.vector.tensor_reduce(out=pk[:, 0:1], in_=st5a[:, 0, :],
                                        axis=AX.X, op=ALU.add)
                nc.vector.tensor_reduce(out=pk[:, 1:2], in_=st5b[:, 0, :],
                                        axis=AX.X, op=ALU.add)
                nc.sync.dma_start(out=cc5i[0][:], in_=pk[:])
                nc.gpsimd.collective_compute(
                    "AllReduce", ALU.add,
                    replica_groups=[list(range(NCORES))],
                    ins=[cc5i[0][:].opt()], outs=[cc5o[0][:].opt()])
                nc.sync.dma_start(out=g5[:, 0], in_=cc5o[0][:])
            elif m == 3:    # AR batch 2: m1..m3
                pk = stat_pool.tile([128, 6], F32, name="pk5b")
                for j, mm in enumerate((1, 2, 3)):
                    nc.vector.tensor_reduce(out=pk[:, 2 * j:2 * j + 1],
                                            in_=st5a[:, mm, :],
                                            axis=AX.X, op=ALU.add)
                    nc.vector.tensor_reduce(out=pk[:, 2 * j + 1:2 * j + 2],
                                            in_=st5b[:, mm, :],
                                            axis=AX.X, op=ALU.add)
                nc.sync.dma_start(out=cc5i[1][:], in_=pk[:])
                nc.gpsimd.collective_compute(
                    "AllReduce", ALU.add,
                    replica_groups=[list(range(NCORES))],
                    ins=[cc5i[1][:].opt()], outs=[cc5o[1][:].opt()])
                nc.sync.dma_start(
                    out=g5[:, 1:4].rearrange("p a b -> p (a b)"),
                    in_=cc5o[1][:])

        def bn_coeffs(gtile, h, aa, dd, pool, tag):
            # gtile[:, h] = [2m, (sum, sumsq)] -> aa/dd [128, 2]
            ms = pool.tile([128, 2, 2], F32, tag=f"{tag}ms", name=f"{tag}ms{h}")
            nc.scalar.activation(
                out=ms[:].rearrange("p a b -> p (a b)"),
                in_=gtile[:, h].rearrange("p a b -> p (a b)"),
                func=AF.Copy, bias=0.0, scale=INV_N)
            var = pool.tile([128, 2], F32, tag=f"{tag}v", name=f"{tag}v{h}")
            nc.vector.tensor_mul(out=var[:], in0=ms[:, :, 0], in1=ms[:, :, 0])
            nc.vector.tensor_sub(out=var[:], in0=ms[:, :, 1], in1=var[:])
            std = pool.tile([128, 2], F32, tag=f"{tag}s", name=f"{tag}s{h}")
            nc.scalar.activation(out=std[:], in_=var[:], func=AF.Sqrt,
                                 bias=eps_sb[:, 0:1], scale=1.0)
            nc.vector.reciprocal(out=std[:], in_=std[:])
            nc.vector.tensor_mul(out=aa[:], in0=std[:],
                                 in1=gam_sb[:, 2 * h:2 * h + 2])
            nc.vector.tensor_mul(out=dd[:], in0=ms[:, :, 0], in1=aa[:])
            nc.vector.tensor_sub(out=dd[:], in0=bet_sb[:, 2 * h:2 * h + 2],
                                 in1=dd[:])

        # ---- BN5 -> a5_t[g] x4 (padded) ----
        def bn_coeff_batch(gsl, gam, bet, nm, pool, tag):
            # gsl: [128, nm, 2] global (sum, sumsq); -> aa, dd [128, nm]
            ms = pool.tile([128, nm, 2], F32, name=f"{tag}ms")
            nc.scalar.activation(
                out=ms[:].rearrange("p a b -> p (a b)"),
                in_=gsl.rearrange("p a b -> p (a b)"),
                func=AF.Copy, bias=0.0, scale=INV_N)
            var = pool.tile([128, nm], F32, name=f"{tag}v")
            nc.vector.tensor_mul(out=var[:], in0=ms[:, :, 0], in1=ms[:, :, 0])
            nc.vector.tensor_sub(out=var[:], in0=ms[:, :, 1], in1=var[:])
            std = pool.tile([128, nm], F32, name=f"{tag}s")
            nc.scalar.activation(out=std[:], in_=var[:], func=AF.Sqrt,
                                 bias=eps_sb[:, 0:1], scale=1.0)
            nc.vector.reciprocal(out=std[:], in_=std[:])
            aa = pool.tile([128, nm], F32, name=f"{tag}aa")
            dd = pool.tile([128, nm], F32, name=f"{tag}dd")
            nc.vector.tensor_mul(out=aa[:], in0=std[:], in1=gam)
            nc.vector.tensor_mul(out=dd[:], in0=ms[:, :, 0], in1=aa[:])
            nc.vector.tensor_sub(out=dd[:], in0=bet, in1=dd[:])
            return aa, dd

        # m0: coeffs+apply gated only on AR1 (lands mid-conv5)
        aa0, dd0 = bn_coeff_batch(g5[:, 0:1], gam_sb[:, 0:1], bet_sb[:, 0:1],
                                  1, stat_pool, "b5m0")
        nc.scalar.activation(
            out=a5_t[0][:, :, 1:9, 1:33], in_=c5_t[0][:],
            func=AF.Identity, bias=dd0[:, 0:1], scale=aa0[:, 0:1])
        # m1..3: batched coeffs after AR2; applies split across engines
        aa3, dd3 = bn_coeff_batch(g5[:, 1:4], gam_sb[:, 1:4], bet_sb[:, 1:4],
                                  3, stat_pool, "b5m123")
        nc.scalar.activation(
            out=a5_t[1][:, 0:4, 1:9, 1:33], in_=c5_t[1][:, 0:4],
            func=AF.Identity, bias=dd3[:, 0:1], scale=aa3[:, 0:1])
        nc.scalar.activation(
            out=a5_t[2][:, :, 1:9, 1:33], in_=c5_t[2][:],
            func=AF.Identity, bias=dd3[:, 1:2], scale=aa3[:, 1:2])
        nc.scalar.activation(
            out=a5_t[1][:, 4:8, 1:9, 1:33], in_=c5_t[1][:, 4:8],
            func=AF.Identity, bias=dd3[:, 0:1], scale=aa3[:, 0:1])
        nc.scalar.activation(
            out=a5_t[3][:, :, 1:9, 1:33], in_=c5_t[3][:],
            func=AF.Identity, bias=dd3[:, 2:3], scale=aa3[:, 2:3])
        pclose(c5_pool)
        if debug:
            for m in range(4):
                for b in range(B):
                    nc.sync.dma_start(
                        out=_ap(dbg["a5"], m * 2048 + b * 256,
                                [[8192, 128], [32, 8], [1, 32]]),
                        in_=a5_t[m][:, b, 1:9, 1:33])

        # ---- conv6: m0/m1 g0 prework covers BN5-AR2; early BN6 m0-2 ----
        pclose(ps56)
        ps6 = popen("ps6", 1, space="PSUM")
        c6_pool = popen("c6", 4, side="left")
        c6_t = [c6_pool.tile([128, B, 8, 32], F16, tag="c6", name=f"c6_{m}")
                for m in range(4)]
        st6a = stat_pool.tile([128, 4, 4], F32)
        st6b = stat_pool.tile([128, 4, 4], F32)
        cc6i = [dram.tile([128, 6 if h == 0 else 2], F32, tag=f"cc6i{h}",
                          name=f"cc6i{h}") for h in range(2)]
        cc6o = [dram.tile([128, 6 if h == 0 else 2], F32, tag=f"cc6o{h}",
                          name=f"cc6o{h}") for h in range(2)]
        g6 = stat_pool.tile([128, 4, 2], F32)
        pts6 = {}

        def c6_alloc(m):
            pts6[m] = [ps6.tile([128, 2, 8, 32], F32, tag="p6", bufs=8,
                                name=f"p6_{m}_{bp}") for bp in range(4)]

        def c6_mm(m, g):
            for tap in range(9):
                kh, kw = tap // 3, tap % 3
                ii = g * 9 + tap
                for bp in range(4):
                    b0 = 2 * bp
                    nc.tensor.matmul(
                        pts6[m][bp][:],
                        w6_sb[:, g, tap, 128 * m:128 * m + 128],
                        a5_t[g][:, b0:b0 + 2, kh:kh + 8, kw:kw + 32],
                        start=(ii == 0), stop=(ii == 35))

        def c6_drain(m):
            for bp in range(4):
                b0 = 2 * bp
                nc.scalar.activation(
                    out=c6_t[m][:, b0:b0 + 2, :, :], in_=pts6[m][bp][:],
                    func=AF.Identity, bias=b6_sb[:, m:m + 1], scale=1.0,
                    accum_out=st6a[:, m, bp:bp + 1])
                scr = scr_pool.tile([128, 512], F16, tag="scr")
                nc.scalar.activation(
                    out=scr[:], in_=pts6[m][bp][:].rearrange(
                        "p a b c -> p (a b c)"),
                    func=AF.Square, bias=b6_sb[:, m:m + 1], scale=1.0,
                    accum_out=st6b[:, m, bp:bp + 1])
            # pool6: w-pairs of even h rows (pre-BN, gamma>0)
            c6v = c6_t[m][:].rearrange("p b (h two) (w v) -> p b h two w v",
                                       two=2, v=2)
            nc.vector.tensor_max(out=c6p_t[m][:, :, :, 0:16],
                                 in0=c6v[:, :, :, 0, :, 0],
                                 in1=c6v[:, :, :, 0, :, 1])

        c6_alloc(0)
        c6_mm(0, 0)
        c6_alloc(1)
        c6_mm(1, 0)
        for g in (1, 2, 3):
            c6_mm(0, g)
        c6_drain(0)
        for g in (1, 2, 3):
            c6_mm(1, g)
        c6_drain(1)
        c6_alloc(2)
        for g in range(4):
            c6_mm(2, g)
        c6_drain(2)
        # AR batch 1: m0..m2
        pk6a = stat_pool.tile([128, 6], F32, name="pk6a")
        for j in range(3):
            nc.vector.tensor_reduce(out=pk6a[:, 2 * j:2 * j + 1],
                                    in_=st6a[:, j, :], axis=AX.X, op=ALU.add)
            nc.vector.tensor_reduce(out=pk6a[:, 2 * j + 1:2 * j + 2],
                                    in_=st6b[:, j, :], axis=AX.X, op=ALU.add)
        nc.sync.dma_start(out=cc6i[0][:], in_=pk6a[:])
        nc.gpsimd.collective_compute(
            "AllReduce", ALU.add,
            replica_groups=[list(range(NCORES))],
            ins=[cc6i[0][:].opt()], outs=[cc6o[0][:].opt()])
        nc.sync.dma_start(
            out=g6[:, 0:3].rearrange("p a b -> p (a b)"), in_=cc6o[0][:])
        # BN6 coeffs+applies for m0-2, emitted before m3 mms (run under them)
        aa6a, dd6a = bn_coeff_batch(g6[:, 0:3], gam_sb[:, 0:3],
                                    bet_sb[:, 0:3], 3, stat_pool, "b6a")
        for mm_ in range(3):
            nc.scalar.activation(
                out=c6p_t[mm_][:], in_=c6p_t[mm_][:], func=AF.Identity,
                bias=dd6a[:, mm_:mm_ + 1], scale=aa6a[:, mm_:mm_ + 1])
        c6_alloc(3)
        for g in range(4):
            c6_mm(3, g)
        c6_drain(3)
        # AR batch 2: m3 alone
        pk6b = stat_pool.tile([128, 2], F32, name="pk6b")
        nc.vector.tensor_reduce(out=pk6b[:, 0:1], in_=st6a[:, 3, :],
                                axis=AX.X, op=ALU.add)
        nc.vector.tensor_reduce(out=pk6b[:, 1:2], in_=st6b[:, 3, :],
                                axis=AX.X, op=ALU.add)
        nc.sync.dma_start(out=cc6i[1][:], in_=pk6b[:])
        nc.gpsimd.collective_compute(
            "AllReduce", ALU.add,
            replica_groups=[list(range(NCORES))],
            ins=[cc6i[1][:].opt()], outs=[cc6o[1][:].opt()])
        nc.sync.dma_start(out=g6[:, 3], in_=cc6o[1][:])
        pclose(a5_pool)
        pclose(c6_pool)
        aa6b, dd6b = bn_coeff_batch(g6[:, 3:4], gam_sb[:, 3:4],
                                    bet_sb[:, 3:4], 1, stat_pool, "b6b")
        nc.scalar.activation(
            out=c6p_t[3][:], in_=c6p_t[3][:], func=AF.Identity,
            bias=dd6b[:, 0:1], scale=aa6b[:, 0:1])
        pclose(scr_pool)
        if debug:
            for g in range(4):
                for b in range(B):
                    nc.sync.dma_start(
                        out=_ap(dbg["c6p"], g * 512 + b * 64,
                                [[2048, 128], [16, 4], [1, 16]]),
                        in_=c6p_t[g][:, b, :, 0:16])

        # ---- conv7 (VALID 2x2, g-outer) -> c7 [128, 4, B, 3, 16] ----
        pclose(ps6)
        ps7 = popen("ps7", 4, space="PSUM")
        c7_pool = popen("c7", 1, side="left")
        c7 = c7_pool.tile([128, 4, B, 3, 16], F16)
        pts7 = [ps7.tile([128, 8, 3, 16], F32, tag="p7", name=f"p7_{m}")
                for m in range(4)]
        for g in range(4):
            for tap in range(4):
                kh, kw = tap // 2, tap % 2
                i = g * 4 + tap
                for m in range(4):
                    nc.tensor.matmul(
                        pts7[m][:], w7_sb[:, g, tap, 128 * m:128 * m + 128],
                        c6p_t[g][:, :, kh:kh + 3, kw:kw + 16],
                        start=(i == 0), stop=(i == 15))
        for m in range(4):
            nc.scalar.activation(out=c7[:, m], in_=pts7[m][:],
                                 func=AF.Identity, bias=b7_sb[:, m:m + 1],
                                 scale=1.0)
        pclose(wD)
        pclose(c6p_pool)
        if debug:
            nc.sync.dma_start(
                out=_ap(dbg["c7"], 0, [[1536, 128], [1, 1536]]),
                in_=c7[:].rearrange("p g b h w -> p (g b h w)"))

        # ---- LSTM ----
        pclose(ps7)
        psL = popen("psL", 1, space="PSUM")
        ls = popen("ls", 1, side="right")
        xg = ls.tile([128, 2, 8, 8, 15], F32)
        hs_d = [ls.tile([128, 2, 8, 15], F16, name=f"hs{dr}")
                for dr in range(2)]
        cst_d = [ls.tile([128, 2, 8], F32, name=f"cst{dr}") for dr in range(2)]
        gp = popen("gp", 4, side="right")
        tp = popen("tp", 6, side="right")

        for dr in range(2):
            for m in range(8):
                pt = psL.tile([128, 8, 16], F32, tag="px", bufs=2)
                for gd in range(8):
                    nc.tensor.matmul(
                        pt[:], wih_sb[:, dr, gd, 128 * m:128 * m + 128],
                        c7[:, gd % 4, :, gd // 4, :],
                        start=(gd == 0), stop=(gd == 7))
                nc.scalar.activation(
                    out=xg[:, dr, m],
                    in_=pt[:, :, 0:15],
                    func=AF.Identity, bias=lb_sb[:, dr, m:m + 1], scale=1.0)
        if debug:
            nc.sync.dma_start(
                out=_ap(dbg["xg"], 0, [[1920, 128], [1, 1920]]),
                in_=xg[:].rearrange("p d m b t -> p (d m b t)"))

        for t in range(15):
            gas = {}
            for dr in range(2):
                tt = t if dr == 0 else 14 - t
                ga = gp.tile([128, 8, 8], F32, tag="ga", name=f"ga{dr}")
                gas[dr] = ga
                if t == 0:
                    nc.vector.tensor_copy(out=ga[:], in_=xg[:, dr, :, :, tt])
                else:
                    tprev = tt - 1 if dr == 0 else tt + 1
                    pr = psL.tile([128, 8, 8], F32, tag=f"pr{dr}", bufs=2,
                                  name=f"pr{dr}")
                    for m in range(8):
                        for gh in range(2):
                            nc.tensor.matmul(
                                pr[:, m, :],
                                whh_sb[:, dr, gh, 128 * m:128 * m + 128],
                                hs_d[dr][:, gh, :, tprev],
                                start=(gh == 0), stop=(gh == 1),
                                skip_group_check=True)
                    nc.vector.tensor_add(out=ga[:], in0=pr[:],
                                         in1=xg[:, dr, :, :, tt])
            for dr in range(2):
                tt = t if dr == 0 else 14 - t
                ga = gas[dr]
                nc.scalar.activation(out=ga[:, 0:6, :], in_=ga[:, 0:6, :],
                                     func=AF.Sigmoid, bias=0.0, scale=1.0)
                nc.scalar.activation(out=ga[:, 6:8, :], in_=ga[:, 6:8, :],
                                     func=AF.Tanh, bias=0.0, scale=1.0)
                cs = cst_d[dr][:]
                if t == 0:
                    nc.vector.tensor_mul(out=cs, in0=ga[:, 0:2, :],
                                         in1=ga[:, 6:8, :])
                else:
                    t2 = tp.tile([128, 2, 8], F32, tag="t2", name=f"t2_{dr}")
                    nc.vector.tensor_mul(out=t2[:], in0=ga[:, 2:4, :], in1=cs)
                    t1 = tp.tile([128, 2, 8], F32, tag="t1", name=f"t1_{dr}")
                    nc.vector.tensor_mul(out=t1[:], in0=ga[:, 0:2, :],
                                         in1=ga[:, 6:8, :])
                    nc.vector.tensor_add(out=cs, in0=t1[:], in1=t2[:])
                th = tp.tile([128, 2, 8], F32, tag="th", name=f"th_{dr}")
                nc.scalar.activation(out=th[:], in_=cs, func=AF.Tanh,
                                     bias=0.0, scale=1.0)
                nc.vector.tensor_mul(out=hs_d[dr][:, :, :, tt],
                                     in0=ga[:, 4:6, :], in1=th[:])

        if debug:
            for dr in range(2):
                nc.sync.dma_start(
                    out=_ap(dbg["hs"], dr * 240, [[480, 128], [1, 240]]),
                    in_=hs_d[dr][:].rearrange("p g b t -> p (g b t)"))

        # ---- output: PE-transpose h to (b*t)-partitions, contiguous DMA ----
        ptt = psL.tile([120, 512], F16, tag="pt_out")
        for dr in range(2):
            for gh in range(2):
                nc.tensor.transpose(
                    ptt[:, 128 * (2 * dr + gh):128 * (2 * dr + gh) + 128],
                    hs_d[dr][:, gh].rearrange("p b t -> p (b t)"),
                    ident16[:])
        outsb = ls.tile([120, 512], F16)
        nc.scalar.copy(out=outsb[:], in_=ptt[:])
        nc.sync.dma_start(out=_ap(out, 0, [[512, 60], [1, 512]]),
                          in_=outsb[0:60])
        nc.scalar.dma_start(out=_ap(out, 60 * 512, [[512, 60], [1, 512]]),
                            in_=outsb[60:120])

        for p in reversed(list(opened)):
            pclose(p)

    nc.compile()
    return nc


def prep_inputs(inputs, core):
    """Host-side: shard + transform weights for one core."""
    d = {}
    x = np.asarray(inputs["x"], dtype=np.float32)
    xs = x[core * B:(core + 1) * B, 0]          # (8, 64, 256)
    xp = np.zeros((B, 66, 258), np.float32)
    xp[:, 1:65, 1:257] = xs
    # xim19: c<9: tap c at x; 9<=c<18: tap c-9 at x+128; c=18: ones
    xim = np.empty((B, 19, 64, 128), np.float32)
    for kh in range(3):
        for kw in range(3):
            xim[:, kh * 3 + kw] = xp[:, kh:kh + 64, kw:kw + 128]
            xim[:, 9 + kh * 3 + kw] = xp[:, kh:kh + 64, 128 + kw:256 + kw]
    xim[:, 18] = 1.0
    d["xim"] = xim.astype(np.float16)

    w1 = np.asarray(inputs["w1"], np.float32)   # (64,1,3,3)
    w1f = w1[:, 0].reshape(64, 9).T             # (9, 64)
    b1 = np.asarray(inputs["b1"], np.float32)
    w1x = np.zeros((19, 128), np.float32)
    w1x[0:9, 0:64] = w1f
    w1x[9:18, 64:128] = w1f
    w1x[18, 0:64] = b1
    w1x[18, 64:128] = b1
    d["w1x"] = w1x.astype(np.float16)

    w2 = np.asarray(inputs["w2"], np.float32)   # (128,64,3,3)
    w2p = np.zeros((3, 128, 128), np.float32)
    for kw in range(3):
        w2p[kw, 0:64] = w2[:, :, 0, kw].T
        w2p[kw, 64:128] = w2[:, :, 1, kw].T
    d["w2p"] = w2p.astype(np.float16)
    d["w2s"] = np.ascontiguousarray(
        np.transpose(w2[:, :, 2, :], (2, 1, 0))).astype(np.float16)  # (3, 64, 128)
    d["b2"] = np.asarray(inputs["b2"], np.float32).reshape(1, 128).T.copy()

    def wT(w, gK, cout):
        o, i_, kh, kw = w.shape
        r = np.transpose(w, (2, 3, 1, 0)).reshape(kh * kw, gK, 128, o)
        return np.ascontiguousarray(np.transpose(r, (1, 0, 2, 3))).astype(np.float16)

    d["w3T"] = wT(np.asarray(inputs["w3"], np.float32), 1, 256)
    d["w4T"] = wT(np.asarray(inputs["w4"], np.float32), 2, 256)
    d["w5T"] = wT(np.asarray(inputs["w5"], np.float32), 2, 512)
    d["w6T"] = wT(np.asarray(inputs["w6"], np.float32), 4, 512)
    w7 = np.asarray(inputs["w7"], np.float32)   # (512,512,2,2)
    r7 = np.transpose(w7, (2, 3, 1, 0)).reshape(4, 4, 128, 512)
    d["w7T"] = np.ascontiguousarray(np.transpose(r7, (1, 0, 2, 3))).astype(np.float16)
    for k, g in (("b3", 2), ("b4", 2), ("b5", 4), ("b6", 4), ("b7", 4)):
        d[k] = np.ascontiguousarray(
            np.asarray(inputs[k], np.float32).reshape(g, 128).T)
    d["gam"] = np.ascontiguousarray(
        np.asarray(inputs["gamma"], np.float32).reshape(4, 128).T)
    d["bet"] = np.ascontiguousarray(
        np.asarray(inputs["beta"], np.float32).reshape(4, 128).T)

    # LSTM: d-column permutation dmap maps compute-chunk col 128*j+p to
    # reference D index 2*(128*(j%4)+p) + j//4
    j = np.arange(8)[:, None]
    p = np.arange(128)[None, :]
    dmap = (2 * (128 * (j % 4) + p) + j // 4).reshape(-1)
    wih = np.stack([np.asarray(inputs["Wih_f"], np.float32),
                    np.asarray(inputs["Wih_b"], np.float32)])
    whh = np.stack([np.asarray(inputs["Whh_f"], np.float32),
                    np.asarray(inputs["Whh_b"], np.float32)])
    wihp = wih[:, PERM4H][:, :, dmap]           # (2, 1024, 1024)
    d["wihT"] = np.ascontiguousarray(
        np.transpose(wihp, (0, 2, 1)).reshape(2, 8, 128, 1024)).astype(np.float16)
    whhp = whh[:, PERM4H]                       # (2, 1024, 256)
    d["whhT"] = np.ascontiguousarray(
        np.transpose(whhp, (0, 2, 1)).reshape(2, 2, 128, 1024)).astype(
            np.float16)
    lb = (np.stack([np.asarray(inputs["bih_f"], np.float32),
                    np.asarray(inputs["bih_b"], np.float32)])
          + np.stack([np.asarray(inputs["bhh_f"], np.float32),
                      np.asarray(inputs["bhh_b"], np.float32)]))
    lbp = lb[:, PERM4H].reshape(2, 8, 128)      # (dir, m, p)
    d["lbias"] = np.ascontiguousarray(np.transpose(lbp, (2, 0, 1)))
    return d


_NC_CACHE = {}


def kernel(**inputs):
    key = "debug" if inputs.pop("_debug", False) else "main"
    if key not in _NC_CACHE:
        _NC_CACHE[key] = build(debug=(key == "debug"))
    nc = _NC_CACHE[key]
    in_maps = [prep_inputs(inputs, c) for c in range(NCORES)]
    res = bass_utils.run_bass_kernel_spmd(nc, in_maps,
                                          core_ids=list(range(NCORES)))
    out = np.concatenate(
        [np.asarray(res.results[c]["out"]).astype(np.float32)
         for c in range(NCORES)], axis=0)
    kernel.last_results = res
    return out

